# revision 32
# baseline (speedup 1.0000x reference)
"""EGConv + GraphNorm + ReLU Trainium2 kernel (8 NeuronCores, SPMD, v2.5).

Strategy (hardcoded for N=100000, E=3200000, D=128, H=8, B=4, A=['sum','max'],
G=64 graphs):
  - Nodes partitioned across 8 cores at graph boundaries (GraphNorm stays
    core-local); no collectives — each core computes the FULL bf16 bases
    table itself, split into two DRAM halves so class-0 gathers can start
    before the whole table is built (beats the serialized AllGather).
  - Table: bf16 bases, pair-packed 256B rows ([bases(2j)|bases(2j+1)]),
    host-reordered class-major (node%4 -> class=(n%4)//2, half=n%2) so
    each of the 2 gather classes is a contiguous [25600, 128]-bf16 region
    addressable by int16 quad ids (node//4).
  - SWDGE dma_gather in 1024-descriptor single_packet calls (the HW limit;
    bigger or multi-packet calls crash / crawl at ~300ns per descriptor).
  - Per dst node, incoming edges are grouped by src%4 = (class, half):
    class = which gather call region, half = which 64-value half of the
    fetched pair holds the src's bases.  Window widths equalized across
    classes (wL, wH) and across cores (SPMD: one program) so sum/max each
    need only two strided 4D-AP reduces (axis=XY over [p, e, class, col]).
  - Pad slots point at dedicated -BIG rows: max works directly; sum is
    corrected by a precomputed pad_count*BIG bias (Act engine).
  - Self loops never gathered: local bases (computed dst-sorted in phase
    A-local) are combined directly on DVE.
  - bf16 messages; comb/einsum in bf16; h0/hsq bf16; stats + GraphNorm
    P/Q in f32; output bf16 (host converts).
  - A tiny warmup gather preloads the gpsimd gather library; it lives in
    the LAST tile pool because shifting the msg/idx pool bases off 256B
    alignment costs ~35% gather bandwidth (every descriptor write then
    straddles two SBUF lines).
  - Engine budget per core (measured): Pool desc-gen ~1.65ms is the
    floor-setter (628 calls x ~0.7us fixed + 566K descriptors x ~2.1ns),
    DMA ~1.5ms (random 256B reads ~37ns/desc/engine), DVE ~1.5ms.
"""

import os
import numpy as np
import ml_dtypes

BF16 = np.dtype(ml_dtypes.bfloat16)

# ---------------- problem constants (hardcoded per spec) ----------------
N = 100000
E = 3200000
D = 128
H = 8
FH = 16         # per-head dim
G = 64          # graphs
EPS = 1e-5
P = 128
NCORES = 8
BF = 64         # bases feature dim
BIG = 64.0      # pad sentinel (exact in bf16)
GPAD = 16       # padded per-core graph count
NTAB = 102400   # padded global table rows (mult of 512)
PADQ = N // 4   # 25000: quad id of the -BIG pad rows (nodes 100000..100003)
COLCAP = 8      # max gather columns per call (8*128 = 1024 descriptors)
SCRATCH = 32768  # dynamic_dma_scratch_size -> 2048-desc SWDGE FIFO

_F32 = np.float32


def _ceil(a, b):
    return -(-a // b) * b


# ======================================================================
# host preprocessing (index/layout only — no float math on features)
# ======================================================================
def _prep(edge_index, batch_ptr):
    counts = np.bincount(batch_ptr, minlength=G).astype(np.int64)
    gcum = np.concatenate([[0], np.cumsum(counts)])  # [G+1]

    # core boundaries at graph boundaries, close to N/8 multiples
    gb = [0]
    for c in range(1, NCORES):
        tgt = N * c / NCORES
        g = int(np.argmin(np.abs(gcum - tgt)))
        g = min(max(g, gb[-1]), G - (NCORES - c))
        gb.append(g)
    gb.append(G)
    node0 = np.array([gcum[gb[c]] for c in range(NCORES)], np.int64)
    ncs = np.array([gcum[gb[c + 1]] - gcum[gb[c]] for c in range(NCORES)],
                   np.int64)
    NMAX = _ceil(int(ncs.max()), P)
    ntiles = NMAX // P

    src_g = np.asarray(edge_index[0], np.int64)
    dst_g = np.asarray(edge_index[1], np.int64)
    bounds = np.concatenate([node0, [N]])
    node_core = np.searchsorted(bounds, np.arange(N), side="right") - 1
    grp_src = src_g & 3          # group = src original id mod 4
    quad_src = src_g >> 2        # int16 gather index (pair-quad id)

    cores = []
    for c in range(NCORES):
        n_c = int(ncs[c])
        m = node_core[dst_g] == c
        es_q = quad_src[m]
        eg = grp_src[m]
        ed = dst_g[m] - node0[c]

        # per-(dst, group) counts; sort dsts desc by (max_g k_g, k0..k3)
        kmat = np.bincount(ed * 4 + eg, minlength=n_c * 4).reshape(n_c, 4)
        kmx = kmat.max(axis=1)
        order = np.lexsort((-kmat[:, 3], -kmat[:, 2], -kmat[:, 1],
                            -kmat[:, 0], -kmx))
        pos_of = np.empty(n_c, np.int64)
        pos_of[order] = np.arange(n_c)

        # CSR over (sorted dst position, group)
        dpos = pos_of[ed]
        key = dpos * 4 + eg
        eorder = np.argsort(key, kind="stable")
        csr_data = es_q[eorder]                       # quad ids, < 25600
        kflat = np.bincount(key, minlength=NMAX * 4)
        indptr = np.concatenate([[0], np.cumsum(kflat)])
        ks = kflat.reshape(NMAX, 4)                   # sorted space

        # graph id per sorted position -> GraphNorm indicators
        gnode = np.searchsorted(gcum, node0[c] + order, side="right") - 1 \
            - gb[c]
        gid = np.full(NMAX, -1, np.int64)
        gid[:n_c] = gnode
        ind = np.zeros((P, ntiles * GPAD), _F32)
        indT = np.zeros((GPAD, ntiles * P), _F32)
        for t in range(ntiles):
            gl = gid[t * P:(t + 1) * P]
            valid = gl >= 0
            pidx = np.arange(P)[valid]
            gv = gl[valid]
            ind[pidx, t * GPAD + gv] = 1.0
            indT[gv, t * P + pidx] = 1.0

        cnt_loc = counts[gb[c]:gb[c + 1]].astype(_F32)
        cntinv = np.zeros(GPAD, _F32)
        cntinv[:len(cnt_loc)] = 1.0 / np.maximum(cnt_loc, 1.0)

        cores.append(dict(
            n=n_c, node0=int(node0[c]), perm=order, pos_of=pos_of,
            csr_data=csr_data, indptr=indptr, ks=ks, ind=ind, indT=indT,
            cntinv=cntinv))

    # ---- GLOBAL (SPMD-shared) per-tile equalized windows ----
    wg = np.zeros((ntiles, 4), np.int64)
    for c in range(NCORES):
        kt = cores[c]["ks"].reshape(ntiles, P, 4)
        wg = np.maximum(wg, kt.max(axis=1))
    wLs = np.maximum(wg[:, 0], wg[:, 2])
    wHs = np.maximum(wg[:, 1], wg[:, 3])

    tiles = []
    s16base = 0
    for t in range(ntiles):
        wl, wh = int(wLs[t]), int(wHs[t])
        wcls = wl + wh
        calls = []   # (cls, col0_in_class, ncols, s16off)
        s16 = 0
        for cls in range(2):
            done = 0
            while done < wcls:
                nc_ = min(COLCAP, wcls - done)
                calls.append((cls, done, nc_, s16))
                s16 += nc_ * 8
                done += nc_
        tiles.append(dict(wl=wl, wh=wh, calls=calls, s16=s16,
                          s16base=s16base))
        s16base += s16
    S16TOT = max(s16base, 8)

    # ---- per-core idx16 + kbig in the global layout ----
    for c in range(NCORES):
        cc = cores[c]
        ks, indptr = cc["ks"], cc["indptr"]
        data = cc["csr_data"] if len(cc["csr_data"]) else np.zeros(1,
                                                                   np.int64)
        idx16 = np.zeros((P, S16TOT), np.int16)
        for t in range(ntiles):
            ti = tiles[t]
            wl = ti["wl"]
            dp = t * P + np.arange(P)
            for (cls, col0, ncols, s16off) in ti["calls"]:
                cols = col0 + np.arange(ncols)
                gsel = np.where(cols < wl, 2 * cls, 2 * cls + 1)
                jj = np.where(cols < wl, cols, cols - wl)
                cnt = ks[dp[:, None], gsel[None, :]]        # [P, ncols]
                start = indptr[dp[:, None] * 4 + gsel[None, :]]
                gidx = np.minimum(start + jj[None, :], len(data) - 1)
                vals = data[gidx]
                mat = np.where(jj[None, :] < cnt, vals, PADQ)  # [P, ncols]
                flat = mat.T.reshape(-1)                       # i = col*128+p
                wrapped = flat.reshape(-1, 16).T.astype(np.int16)
                idx16[:, ti["s16base"] + s16off:
                      ti["s16base"] + s16off + ncols * 8] = np.tile(
                          wrapped, (8, 1))
        cc["idx16"] = idx16

        # sum pad-correction: pads = 2*(wl+wh) - ktot  ->  +pads*BIG
        ktot = ks.sum(axis=1)
        wtot_n = (2 * (wLs + wHs))[:, None].repeat(P, axis=1).reshape(-1)
        kbig = ((wtot_n - ktot) * BIG).astype(_F32)
        cc["kbig"] = np.ascontiguousarray(
            kbig.reshape(ntiles, P).T)                     # [128, ntiles]

    return dict(cores=cores, NMAX=NMAX, ntiles=ntiles, tiles=tiles,
                S16TOT=S16TOT, node0=node0, ncs=ncs)


def _tabpos(n):
    """Table position of node n: class-major layout so each gather class
    is a contiguous [25600, 128]-bf16 (256B-row) region."""
    return ((n % 4) // 2) * (NTAB // 2) + (n // 4) * 2 + (n % 2)


def _make_inputs(cfg, node, W_bases, W_comb, b_comb, bias_out, gn_weight,
                 gn_bias, gn_mean_scale):
    node = np.asarray(node, _F32)
    NMAX = cfg["NMAX"]
    nodeTF = np.zeros((P, NTAB), BF16)
    nodeTF[:, _tabpos(np.arange(N))] = node.T.astype(BF16)
    wcat = np.concatenate([np.asarray(W_bases, _F32),
                           np.asarray(W_comb, _F32)], axis=1).astype(BF16)
    bcomb = np.asarray(b_comb, _F32).reshape(1, BF)
    gaux0 = np.zeros((GPAD, 520), _F32)
    gaux0[:, 1:129] = np.asarray(bias_out, _F32)[None, :]
    gaux0[:, 129:257] = np.asarray(gn_mean_scale, _F32)[None, :]
    gaux0[:, 257:385] = np.asarray(gn_weight, _F32)[None, :]
    gaux0[:, 385:513] = np.asarray(gn_bias, _F32)[None, :]

    in_maps = []
    for c in range(NCORES):
        cc = cfg["cores"][c]
        nl = np.zeros((NMAX, D), _F32)
        nl[:cc["n"]] = node[cc["node0"]:cc["node0"] + cc["n"]][cc["perm"]]
        ga = gaux0.copy()
        ga[:, 0] = cc["cntinv"]
        in_maps.append({
            "nodeTF": nodeTF,                              # [128, NTAB] bf16
            "nodeTL": np.ascontiguousarray(nl.T).astype(BF16),
            "wcat": wcat,                                  # [128, 128] bf16
            "bcomb": bcomb,                                # [1, 64] f32
            "idx": cc["idx16"],                            # [128, S16TOT]
            "kbig": cc["kbig"],                            # [128, ntiles] f32
            "ind": np.ascontiguousarray(cc["ind"]).astype(BF16),
            "indT": np.ascontiguousarray(cc["indT"]),      # [16, nt*128] f32
            "gaux": ga,                                    # [16, 520] f32
        })
    return in_maps


# ======================================================================
# numpy simulation of the device algorithm (with bf16 quantization)
# ======================================================================
def _numpy_sim(cfg, in_maps):
    outs = []
    ntiles, NMAX = cfg["ntiles"], cfg["NMAX"]
    im0 = in_maps[0]
    nodeTF = im0["nodeTF"].astype(_F32)
    wcat = im0["wcat"].astype(_F32)
    table = (nodeTF.T @ wcat[:, :BF]).astype(BF16)          # [NTAB, 64]
    table[_tabpos(np.arange(N, N + 4))] = BF16.type(-BIG)

    for c in range(NCORES):
        im = in_maps[c]
        nodeTL = im["nodeTL"].astype(_F32)
        full = nodeTL.T @ wcat                              # [NMAX, 128]
        basloc = full[:, :BF].astype(BF16)
        comb = (full[:, BF:] + im["bcomb"][0][None, :]).astype(BF16)

        kbig = im["kbig"].T                                 # [ntiles, 128]
        h0 = np.zeros((NMAX, D), _F32)
        for t in range(ntiles):
            ti = cfg["tiles"][t]
            wl, wh = ti["wl"], ti["wh"]
            wcls = wl + wh
            wtot = 2 * wcls
            msg = np.zeros((P, max(wtot, 1), 128), _F32)
            for (cls, col0, ncols, s16off) in ti["calls"]:
                blk = im["idx"][:16, ti["s16base"] + s16off:
                                ti["s16base"] + s16off + ncols * 8]
                f2 = blk.T.reshape(-1)
                vals = f2[:ncols * 128].astype(np.int64)
                # class c region row r -> table rows (c*NTAB/2 + 2r, +1)
                # = nodes (4r+2c, 4r+2c+1): 128 bf16 payload
                base = cls * (NTAB // 2)
                rows = np.concatenate(
                    [table[base + 2 * vals],
                     table[base + 2 * vals + 1]],
                    axis=1).astype(_F32)                    # [ncols*128,128]
                msg[:, cls * wcls + col0:cls * wcls + col0 + ncols, :] = \
                    rows.reshape(ncols, P, 128).transpose(1, 0, 2)
            mv = msg.reshape(P, 2, -1, 128)[:, :, :wcls or 1, :]
            bl = basloc[t * P:(t + 1) * P].astype(_F32)
            ssum = kbig[t][:, None] + bl
            smax = np.full((P, BF), -BIG, _F32)
            if wl:
                ssum = ssum + mv[:, :, :wl, 0:64].sum(axis=(1, 2))
                smax = np.maximum(smax, mv[:, :, :wl, 0:64].max(axis=(1, 2)))
            if wh:
                ssum = ssum + mv[:, :, wl:wcls, 64:128].sum(axis=(1, 2))
                smax = np.maximum(smax,
                                  mv[:, :, wl:wcls, 64:128].max(axis=(1, 2)))
            smax = np.maximum(smax, bl)
            aggcat = np.concatenate([ssum, smax], axis=1)   # [P, 128]
            cb = comb[t * P:(t + 1) * P].astype(_F32)       # [P, 64]
            prod = (cb.reshape(P, H, 8, 1) *
                    aggcat.reshape(P, 1, 8, FH)).astype(BF16).astype(_F32)
            h0[t * P:(t + 1) * P] = prod.sum(axis=2).reshape(P, D)
        h0q = h0.astype(BF16).astype(_F32)
        hsq = (h0q * h0q).astype(BF16).astype(_F32)

        # graphnorm (f32, as baseline)
        ind = im["ind"].astype(_F32).reshape(P, ntiles, GPAD)
        ga = im["gaux"]
        cntinv = ga[:, 0:1]
        bias_o = ga[:, 1:129]
        ms = ga[:, 129:257]
        gnw = ga[:, 257:385]
        gnb = ga[:, 385:513]
        s1 = np.zeros((GPAD, D), _F32)
        s2 = np.zeros((GPAD, D), _F32)
        for t in range(ntiles):
            s1 += ind[:, t, :].T @ h0q[t * P:(t + 1) * P]
            s2 += ind[:, t, :].T @ hsq[t * P:(t + 1) * P]
        m0 = s1 * cntinv
        mh = m0 + bias_o
        e2 = s2 * cntinv + bias_o * (2 * m0 + bias_o)
        c0 = mh * ms
        var = e2 - 2 * c0 * mh + c0 * c0
        rstd = 1.0 / np.sqrt(var + EPS)
        Pm = gnw * rstd
        Qm = (bias_o - c0) * Pm + gnb
        indT = im["indT"].reshape(GPAD, ntiles, P)
        hfin = np.zeros((NMAX, D), _F32)
        for t in range(ntiles):
            Pn = indT[:, t, :].T @ Pm
            Qn = indT[:, t, :].T @ Qm
            hfin[t * P:(t + 1) * P] = np.maximum(
                h0q[t * P:(t + 1) * P] * Pn + Qn, 0.0)
        outs.append(hfin.astype(BF16))
    return outs


def _assemble(cfg, per_core_h):
    out = np.zeros((N, D), _F32)
    for c in range(NCORES):
        cc = cfg["cores"][c]
        h = np.asarray(per_core_h[c])
        if h.dtype != _F32:
            h = h.astype(_F32)
        out[cc["node0"] + cc["perm"]] = h[:cc["n"]]
    return out


# ======================================================================
# device program
# ======================================================================
def _build(cfg):
    import concourse.bacc as bacc
    import concourse.tile as tile
    from concourse import mybir

    ntiles, NMAX, S16TOT = cfg["ntiles"], cfg["NMAX"], cfg["S16TOT"]
    f32 = mybir.dt.float32
    bf16 = mybir.dt.bfloat16
    i16 = mybir.dt.int16
    ALU = mybir.AluOpType
    ACT = mybir.ActivationFunctionType
    AX = mybir.AxisListType

    nc = bacc.Bacc("TRN2", target_bir_lowering=False, debug=False,
                   num_devices=NCORES, num_swdge_queues=4,
                   dynamic_dma_scratch_size=SCRATCH)

    nodeTF = nc.dram_tensor("nodeTF", [P, NTAB], bf16,
                            kind="ExternalInput").ap()
    nodeTL = nc.dram_tensor("nodeTL", [P, NMAX], bf16,
                            kind="ExternalInput").ap()
    wcat = nc.dram_tensor("wcat", [D, D], bf16, kind="ExternalInput").ap()
    bcomb = nc.dram_tensor("bcomb", [1, BF], f32, kind="ExternalInput").ap()
    idx = nc.dram_tensor("idx", [P, S16TOT], i16, kind="ExternalInput").ap()
    kbig = nc.dram_tensor("kbig", [P, ntiles], f32,
                          kind="ExternalInput").ap()
    ind = nc.dram_tensor("ind", [P, ntiles * GPAD], bf16,
                         kind="ExternalInput").ap()
    indT = nc.dram_tensor("indT", [GPAD, ntiles * P], f32,
                          kind="ExternalInput").ap()
    gaux = nc.dram_tensor("gaux", [GPAD, 520], f32, kind="ExternalInput").ap()
    h_out = nc.dram_tensor("h", [NMAX, D], bf16, kind="ExternalOutput").ap()

    with tile.TileContext(nc) as tc:
        with (
            tc.tile_pool(name="dram", bufs=1, space="DRAM") as dram,
            tc.tile_pool(name="persist", bufs=1) as pp,
            tc.tile_pool(name="work", bufs=3) as wp,
            tc.tile_pool(name="nstream", bufs=2) as nsp,
            tc.tile_pool(name="idxp", bufs=6) as ixp,
            tc.tile_pool(name="msgp", bufs=3) as mp,
            tc.tile_pool(name="psum", bufs=2, space="PSUM") as psp,
            tc.tile_pool(name="psumb", bufs=3, space="PSUM") as psb,
            tc.tile_pool(name="statps", bufs=1, space="PSUM") as stp,
            tc.tile_pool(name="warm", bufs=1) as wmp,
        ):
            tbl0 = dram.tile([NTAB // 2, BF], bf16)
            tbl1 = dram.tile([NTAB // 2, BF], bf16)
            tblh = [tbl0, tbl1]

            # ---- constants / persistent
            wcat_s = pp.tile([D, D], bf16)
            nc.sync.dma_start(wcat_s[:], wcat[:])
            bcomb_s = pp.tile([1, BF], f32)
            nc.sync.dma_start(bcomb_s[:], bcomb[:])
            ones1 = pp.tile([1, P], f32)
            nc.vector.memset(ones1[:], 1.0)
            padt = pp.tile([2, BF], bf16)
            nc.vector.memset(padt[:], -BIG)
            kbig_s = pp.tile([P, ntiles], f32)
            nc.sync.dma_start(kbig_s[:], kbig[:])
            ind_s = pp.tile([P, ntiles * GPAD], bf16)
            nc.sync.dma_start(ind_s[:], ind[:])
            gaux_s = pp.tile([GPAD, 520], f32)
            nc.sync.dma_start(gaux_s[:], gaux[:])

            comb_all = pp.tile([P, ntiles * BF], bf16)
            basloc = pp.tile([P, ntiles * BF], bf16)
            h0_all = pp.tile([P, ntiles * D], bf16)

            # warmup: force the gpsimd gather-library load + queue setup
            # before phase A so the first real gather isn't delayed.
            # NB: lives in the LAST pool so it can't shift the 256B alignment
            # of the msg/idx pools (a 272B shift cost ~35% gather bandwidth).
            wupi = wmp.tile([P, 8], i16)
            nc.vector.memset(wupi[:], 0)
            wupm = wmp.tile([P, 1, 128], bf16)
            nc.gpsimd.dma_gather(
                wupm[:], nodeTF[:1, :].rearrange("o (r f) -> (o r) f", f=128),
                wupi[:], P, P, 128, queue_num=0, single_packet=True)

            # ---- phase A: global bases table with A-local interleaved ----
            # (one local tile per 8 full tiles so comb/basloc are ready
            #  progressively; phase C's DVE chain otherwise stalls ~145us
            #  waiting for them after the table completes)
            NFT = NTAB // P           # 800 table tiles
            AF = 8                    # nodeTF stream chunk
            TW = 8                    # table-write batch (tiles)
            AL = 10                   # nodeTL stream chunk
            for t in range(NFT):
                if t % AF == 0:
                    nfb = nsp.tile([P, AF * P], bf16, tag="nfb")
                    nb = min(AF, NFT - t)
                    nc.sync.dma_start(nfb[:, :nb * P],
                                      nodeTF[:, t * P:(t + nb) * P])
                if t % TW == 0:
                    bw = nsp.tile([P, TW * BF], bf16, tag="bw")
                nt_ = nfb[:, (t % AF) * P:(t % AF + 1) * P]
                psB = psb.tile([P, BF], f32, tag="psB")
                nc.tensor.matmul(psB[:], nt_, wcat_s[:, :BF],
                                 start=True, stop=True)
                # alternate copy engine: Act / DVE (both idle in phase A)
                if t % 2 == 0:
                    nc.scalar.copy(bw[:, (t % TW) * BF:(t % TW + 1) * BF],
                                   psB[:])
                else:
                    nc.vector.tensor_copy(
                        bw[:, (t % TW) * BF:(t % TW + 1) * BF], psB[:])
                if t % TW == TW - 1:
                    t0 = t - (TW - 1)
                    half, hr = divmod(t0 * P, NTAB // 2)
                    out_ap = tblh[half][hr:hr + TW * P, :].rearrange(
                        "(a p) f -> p a f", p=P)
                    nc.sync.dma_start(out_ap, bw[:])
                    if (t + 1) * P == (half + 1) * (NTAB // 2):
                        # half complete: write its -BIG pad rows now so
                        # gathers on this class aren't gated on the rest
                        nc.sync.dma_start(
                            tblh[half][2 * PADQ:2 * PADQ + 2, :], padt[:])
                if t % 8 == 0 and t // 8 < ntiles:
                    tl = t // 8
                    if tl % AL == 0:
                        nlb = nsp.tile([P, AL * P], bf16, tag="nlb")
                        nb = min(AL, ntiles - tl)
                        nc.sync.dma_start(nlb[:, :nb * P],
                                          nodeTL[:, tl * P:(tl + nb) * P])
                    ntl = nlb[:, (tl % AL) * P:(tl % AL + 1) * P]
                    psA = psp.tile([P, D], f32, tag="psA")
                    nc.tensor.matmul(psA[:], ntl, wcat_s[:], start=True,
                                     stop=False)
                    nc.tensor.matmul(psA[:, BF:], ones1[:], bcomb_s[:],
                                     start=False, stop=True)
                    nc.scalar.copy(basloc[:, tl * BF:(tl + 1) * BF],
                                   psA[:, :BF])
                    nc.scalar.copy(comb_all[:, tl * BF:(tl + 1) * BF],
                                   psA[:, BF:])

            # gather source: two contiguous class regions, 256B rows
            tblq = [tblh[c][:].rearrange("(r x) f -> r (x f)", x=2)
                    for c in range(2)]

            # ---------------- phase C: gather + aggregate + einsum --------
            stats = stp.tile([GPAD, 2 * D], f32)
            qrot = 0
            first_mm = True
            for zi, t in enumerate(range(ntiles)):
                ti = cfg["tiles"][t]
                wl, wh = ti["wl"], ti["wh"]
                wcls = wl + wh
                wtot = 2 * wcls
                idxt = ixp.tile([P, max(ti["s16"], 8)], i16, tag="idxt")
                if ti["s16"]:
                    nc.sync.dma_start(
                        idxt[:, :ti["s16"]],
                        idx[:, ti["s16base"]:ti["s16base"] + ti["s16"]])
                msg = mp.tile([P, max(wtot, 1), 128], bf16, tag="msg")
                for (cls, col0, ncols, s16off) in ti["calls"]:
                    nc.gpsimd.dma_gather(
                        msg[:, cls * wcls + col0:cls * wcls + col0 + ncols,
                            :],
                        tblq[cls],
                        idxt[:, s16off:s16off + ncols * 8],
                        ncols * P, ncols * P, 128,
                        queue_num=qrot % 4,
                        single_packet=True,
                    )
                    qrot += 1
                # views: [p, cls, col, elem]; all 4 reduces first so the
                # msg buffer frees as early as possible
                mv = msg[:].rearrange("p (c w) e -> p c w e", c=2)
                aggcat = wp.tile([P, 2 * BF], f32, tag="aggcat")
                asum = wp.tile([P, BF], f32, tag="asum")
                amax = wp.tile([P, BF], f32, tag="amax")
                if wl > 0:
                    nc.vector.tensor_reduce(
                        asum[:],
                        mv[:, :, :wl, 0:BF].rearrange("p c w e -> p e c w"),
                        axis=AX.XY, op=ALU.add)
                    nc.vector.tensor_reduce(
                        amax[:],
                        mv[:, :, :wl, 0:BF].rearrange("p c w e -> p e c w"),
                        axis=AX.XY, op=ALU.max)
                else:
                    nc.vector.memset(asum[:], 0.0)
                    nc.vector.memset(amax[:], -BIG)
                if wh > 0:
                    nc.vector.tensor_reduce(
                        aggcat[:, :BF],
                        mv[:, :, wl:wcls, BF:2 * BF].rearrange(
                            "p c w e -> p e c w"),
                        axis=AX.XY, op=ALU.add)
                    nc.vector.tensor_reduce(
                        aggcat[:, BF:],
                        mv[:, :, wl:wcls, BF:2 * BF].rearrange(
                            "p c w e -> p e c w"),
                        axis=AX.XY, op=ALU.max)
                    nc.vector.tensor_tensor(out=asum[:], in0=asum[:],
                                            in1=aggcat[:, :BF], op=ALU.add)
                    nc.vector.tensor_tensor(out=amax[:], in0=amax[:],
                                            in1=aggcat[:, BF:], op=ALU.max)
                # + pad correction, then + self-loop bases
                nc.scalar.activation(asum[:], asum[:], ACT.Identity,
                                     bias=kbig_s[:, t:t + 1], scale=1.0)
                nc.vector.tensor_tensor(out=aggcat[:, :BF], in0=asum[:],
                                        in1=basloc[:, t * BF:(t + 1) * BF],
                                        op=ALU.add)
                nc.vector.tensor_tensor(out=aggcat[:, BF:], in0=amax[:],
                                        in1=basloc[:, t * BF:(t + 1) * BF],
                                        op=ALU.max)
                # einsum: h0[p, h*16] = sum_k comb[p,h,k] * aggcat[p,k,16]
                prod = wp.tile([P, H, 8, FH], bf16, tag="prod")
                cview = comb_all[:, t * BF:(t + 1) * BF].rearrange(
                    "p (h k) -> p h k", h=H)
                nc.vector.tensor_tensor(
                    out=prod[:],
                    in0=cview.to_broadcast([P, H, 8, FH]),
                    in1=aggcat[:].rearrange("p (k f) -> p k f", k=8)
                    [:, None, :, :].broadcast_to([P, H, 8, FH]),
                    op=ALU.mult)
                h0f = wp.tile([P, D], f32, tag="h0f")
                nc.vector.tensor_reduce(
                    h0f[:], prod[:].rearrange("p h k f -> p h f k"),
                    axis=AX.X, op=ALU.add)
                nc.scalar.copy(h0_all[:, t * D:(t + 1) * D], h0f[:])
                hsq = wp.tile([P, D], bf16, tag="hsq")
                nc.scalar.square(hsq[:], h0f[:])
                nc.tensor.matmul(
                    stats[:, :D], ind_s[:, t * GPAD:(t + 1) * GPAD],
                    h0_all[:, t * D:(t + 1) * D],
                    start=first_mm, stop=(zi == ntiles - 1))
                nc.tensor.matmul(
                    stats[:, D:], ind_s[:, t * GPAD:(t + 1) * GPAD],
                    hsq[:],
                    start=first_mm, stop=(zi == ntiles - 1))
                first_mm = False

            # ---------------- phase D: per-graph P/Q ----------------
            st = pp.tile([GPAD, 2 * D], f32)
            nc.vector.tensor_copy(st[:], stats[:])
            cntinv = gaux_s[:, 0:1]
            bias_o = gaux_s[:, 1:129]
            ms = gaux_s[:, 129:257]
            gnw = gaux_s[:, 257:385]
            gnb = gaux_s[:, 385:513]
            s1 = st[:, :D]
            s2 = st[:, D:]
            m0 = pp.tile([GPAD, D], f32)
            nc.vector.tensor_scalar_mul(m0[:], s1, cntinv)
            mh = pp.tile([GPAD, D], f32)
            nc.vector.tensor_tensor(out=mh[:], in0=m0[:], in1=bias_o,
                                    op=ALU.add)
            t1 = pp.tile([GPAD, D], f32)
            nc.vector.scalar_tensor_tensor(out=t1[:], in0=m0[:], scalar=2.0,
                                           in1=bias_o, op0=ALU.mult,
                                           op1=ALU.add)
            t2 = pp.tile([GPAD, D], f32)
            nc.vector.tensor_tensor(out=t2[:], in0=bias_o, in1=t1[:],
                                    op=ALU.mult)
            e2 = pp.tile([GPAD, D], f32)
            nc.vector.tensor_scalar_mul(e2[:], s2, cntinv)
            nc.vector.tensor_tensor(out=e2[:], in0=e2[:], in1=t2[:],
                                    op=ALU.add)
            c0 = pp.tile([GPAD, D], f32)
            nc.vector.tensor_tensor(out=c0[:], in0=mh[:], in1=ms,
                                    op=ALU.mult)
            t3 = pp.tile([GPAD, D], f32)
            nc.vector.tensor_tensor(out=t3[:], in0=c0[:], in1=mh[:],
                                    op=ALU.mult)
            var = pp.tile([GPAD, D], f32)
            nc.vector.scalar_tensor_tensor(out=var[:], in0=t3[:],
                                           scalar=-2.0, in1=e2[:],
                                           op0=ALU.mult, op1=ALU.add)
            t4 = pp.tile([GPAD, D], f32)
            nc.vector.tensor_tensor(out=t4[:], in0=c0[:], in1=c0[:],
                                    op=ALU.mult)
            nc.vector.tensor_tensor(out=var[:], in0=var[:], in1=t4[:],
                                    op=ALU.add)
            stdv = pp.tile([GPAD, D], f32)
            epsc = pp.tile([GPAD, 1], f32)
            nc.vector.memset(epsc[:], EPS)
            nc.scalar.activation(stdv[:], var[:], ACT.Sqrt, bias=epsc[:],
                                 scale=1.0)
            rstd = pp.tile([GPAD, D], f32)
            nc.vector.reciprocal(rstd[:], stdv[:])
            PQ = pp.tile([GPAD, 2 * D], f32)
            nc.vector.tensor_tensor(out=PQ[:, :D], in0=gnw, in1=rstd[:],
                                    op=ALU.mult)
            t5 = pp.tile([GPAD, D], f32)
            nc.vector.tensor_tensor(out=t5[:], in0=bias_o, in1=c0[:],
                                    op=ALU.subtract)
            nc.vector.tensor_tensor(out=PQ[:, D:], in0=t5[:], in1=PQ[:, :D],
                                    op=ALU.mult)
            nc.vector.tensor_tensor(out=PQ[:, D:], in0=PQ[:, D:], in1=gnb,
                                    op=ALU.add)

            # ------- phase E: normalize + relu + out (2-tile batches) -----
            ET = 10
            EB = 2
            for t in range(0, ntiles, EB):
                if t % ET == 0:
                    itb = nsp.tile([GPAD, ET * P], f32, tag="itb")
                    nb = min(ET, ntiles - t)
                    nc.sync.dma_start(itb[:, :nb * P],
                                      indT[:, t * P:(t + nb) * P])
                pq = psp.tile([P, EB, 2 * D], f32, tag="pq")
                for b in range(EB):
                    nc.tensor.matmul(
                        pq[:, b, :],
                        itb[:, (t % ET + b) * P:(t % ET + b + 1) * P],
                        PQ[:], start=True, stop=True)
                hf = wp.tile([P, EB, D], f32, tag="hf")
                nc.vector.tensor_tensor(
                    out=hf[:],
                    in0=h0_all[:, t * D:(t + EB) * D].rearrange(
                        "p (b d) -> p b d", b=EB),
                    in1=pq[:, :, :D], op=ALU.mult)
                nc.vector.tensor_tensor(out=hf[:], in0=hf[:],
                                        in1=pq[:, :, D:], op=ALU.add)
                ho = wp.tile([P, EB, D], bf16, tag="ho")
                nc.scalar.activation(ho[:], hf[:], ACT.Relu)
                nc.sync.dma_start(
                    h_out[t * P:(t + EB) * P, :].rearrange(
                        "(a p) f -> p a f", p=P),
                    ho[:])

    nc.compile()
    return nc


_CACHE = {}


def kernel(node, edge_index, edge_attr, batch_ptr, W_bases, W_comb, b_comb,
           bias_out, gn_weight, gn_bias, gn_mean_scale):
    node = np.asarray(node)
    edge_index = np.asarray(edge_index)
    batch_ptr = np.asarray(batch_ptr)
    cfg = _prep(edge_index, batch_ptr)
    in_maps = _make_inputs(cfg, node, W_bases, W_comb, b_comb, bias_out,
                           gn_weight, gn_bias, gn_mean_scale)

    if os.environ.get("EGC_NUMPY_SIM"):
        return _assemble(cfg, _numpy_sim(cfg, in_maps))

    from concourse.bass_utils import run_bass_kernel_spmd
    key = "prog"
    if key not in _CACHE:
        _CACHE[key] = _build(cfg)
    nc = _CACHE[key]
    res = run_bass_kernel_spmd(nc, in_maps, core_ids=list(range(NCORES)),
                               **_CACHE.get("run_kwargs", {}))
    _CACHE["last_res"] = res
    return _assemble(cfg, [res.results[c]["h"] for c in range(NCORES)])


# revision 33
# speedup vs baseline: 1.5352x; 1.5352x over previous
"""EGConv + GraphNorm + ReLU Trainium2 kernel (8 NeuronCores, SPMD, v2.5).

Strategy (hardcoded for N=100000, E=3200000, D=128, H=8, B=4, A=['sum','max'],
G=64 graphs):
  - Nodes partitioned across 8 cores at graph boundaries (GraphNorm stays
    core-local); no collectives — each core computes the FULL bf16 bases
    table itself, split into two DRAM halves so class-0 gathers can start
    before the whole table is built (beats the serialized AllGather).
  - Table: bf16 bases, pair-packed 256B rows ([bases(2j)|bases(2j+1)]),
    host-reordered class-major (node%4 -> class=(n%4)//2, half=n%2) so
    each of the 2 gather classes is a contiguous [25600, 128]-bf16 region
    addressable by int16 quad ids (node//4).
  - SWDGE dma_gather in 1024-descriptor single_packet calls (the HW limit;
    bigger or multi-packet calls crash / crawl at ~300ns per descriptor).
  - Per dst node, incoming edges are grouped by src%4 = (class, half):
    class = which gather call region, half = which 64-value half of the
    fetched pair holds the src's bases.  Window widths equalized across
    classes (wL, wH) and across cores (SPMD: one program) so sum/max each
    need only two strided 4D-AP reduces (axis=XY over [p, e, class, col]).
  - Pad slots point at dedicated -BIG rows: max works directly; sum is
    corrected by a precomputed pad_count*BIG bias (Act engine).
  - Self loops never gathered: local bases (computed dst-sorted in phase
    A-local) are combined directly on DVE.
  - bf16 messages; comb/einsum in bf16; h0/hsq bf16; stats + GraphNorm
    P/Q in f32; output bf16 (host converts).
  - A tiny warmup gather preloads the gpsimd gather library; it lives in
    the LAST tile pool because shifting the msg/idx pool bases off 256B
    alignment costs ~35% gather bandwidth (every descriptor write then
    straddles two SBUF lines).
  - Engine budget per core (measured): Pool desc-gen ~1.65ms is the
    floor-setter (628 calls x ~0.7us fixed + 566K descriptors x ~2.1ns),
    DMA ~1.5ms (random 256B reads ~37ns/desc/engine), DVE ~1.5ms.
"""

import os
import numpy as np
import ml_dtypes

BF16 = np.dtype(ml_dtypes.bfloat16)

# ---------------- problem constants (hardcoded per spec) ----------------
N = 100000
E = 3200000
D = 128
H = 8
FH = 16         # per-head dim
G = 64          # graphs
EPS = 1e-5
P = 128
NCORES = 8
BF = 64         # bases feature dim
BIG = 64.0      # pad sentinel (exact in bf16)
GPAD = 16       # padded per-core graph count
NTAB = 102400   # padded global table rows (mult of 512)
PADQ = N // 4   # 25000: quad id of the -BIG pad rows (nodes 100000..100003)
COLCAP = 8      # max gather columns per call (8*128 = 1024 descriptors)
SCRATCH = 32768  # dynamic_dma_scratch_size -> 2048-desc SWDGE FIFO

_F32 = np.float32


def _ceil(a, b):
    return -(-a // b) * b


# ======================================================================
# host preprocessing (index/layout only — no float math on features)
# ======================================================================
def _prep(edge_index, batch_ptr):
    counts = np.bincount(batch_ptr, minlength=G).astype(np.int64)
    gcum = np.concatenate([[0], np.cumsum(counts)])  # [G+1]

    # core boundaries at graph boundaries, close to N/8 multiples
    gb = [0]
    for c in range(1, NCORES):
        tgt = N * c / NCORES
        g = int(np.argmin(np.abs(gcum - tgt)))
        g = min(max(g, gb[-1]), G - (NCORES - c))
        gb.append(g)
    gb.append(G)
    node0 = np.array([gcum[gb[c]] for c in range(NCORES)], np.int64)
    ncs = np.array([gcum[gb[c + 1]] - gcum[gb[c]] for c in range(NCORES)],
                   np.int64)
    NMAX = _ceil(int(ncs.max()), P)
    ntiles = NMAX // P

    src_g = np.asarray(edge_index[0], np.int64)
    dst_g = np.asarray(edge_index[1], np.int64)
    bounds = np.concatenate([node0, [N]])
    node_core = np.searchsorted(bounds, np.arange(N), side="right") - 1
    grp_src = src_g & 3          # group = src original id mod 4
    quad_src = src_g >> 2        # int16 gather index (pair-quad id)

    cores = []
    for c in range(NCORES):
        n_c = int(ncs[c])
        m = node_core[dst_g] == c
        es_q = quad_src[m]
        eg = grp_src[m]
        ed = dst_g[m] - node0[c]

        # per-(dst, group) counts; sort dsts desc by (max_g k_g, k0..k3)
        kmat = np.bincount(ed * 4 + eg, minlength=n_c * 4).reshape(n_c, 4)
        kmx = kmat.max(axis=1)
        order = np.lexsort((-kmat[:, 3], -kmat[:, 2], -kmat[:, 1],
                            -kmat[:, 0], -kmx))
        pos_of = np.empty(n_c, np.int64)
        pos_of[order] = np.arange(n_c)

        # CSR over (sorted dst position, group)
        dpos = pos_of[ed]
        key = dpos * 4 + eg
        eorder = np.argsort(key, kind="stable")
        csr_data = es_q[eorder]                       # quad ids, < 25600
        kflat = np.bincount(key, minlength=NMAX * 4)
        indptr = np.concatenate([[0], np.cumsum(kflat)])
        ks = kflat.reshape(NMAX, 4)                   # sorted space

        # graph id per sorted position -> GraphNorm indicators
        gnode = np.searchsorted(gcum, node0[c] + order, side="right") - 1 \
            - gb[c]
        gid = np.full(NMAX, -1, np.int64)
        gid[:n_c] = gnode
        ind = np.zeros((P, ntiles * GPAD), _F32)
        indT = np.zeros((GPAD, ntiles * P), _F32)
        for t in range(ntiles):
            gl = gid[t * P:(t + 1) * P]
            valid = gl >= 0
            pidx = np.arange(P)[valid]
            gv = gl[valid]
            ind[pidx, t * GPAD + gv] = 1.0
            indT[gv, t * P + pidx] = 1.0

        cnt_loc = counts[gb[c]:gb[c + 1]].astype(_F32)
        cntinv = np.zeros(GPAD, _F32)
        cntinv[:len(cnt_loc)] = 1.0 / np.maximum(cnt_loc, 1.0)

        cores.append(dict(
            n=n_c, node0=int(node0[c]), perm=order, pos_of=pos_of,
            csr_data=csr_data, indptr=indptr, ks=ks, ind=ind, indT=indT,
            cntinv=cntinv))

    # ---- GLOBAL (SPMD-shared) per-tile equalized windows ----
    wg = np.zeros((ntiles, 4), np.int64)
    for c in range(NCORES):
        kt = cores[c]["ks"].reshape(ntiles, P, 4)
        wg = np.maximum(wg, kt.max(axis=1))
    wLs = np.maximum(wg[:, 0], wg[:, 2])
    wHs = np.maximum(wg[:, 1], wg[:, 3])

    tiles = []
    s16base = 0
    for t in range(ntiles):
        wl, wh = int(wLs[t]), int(wHs[t])
        wcls = wl + wh
        calls = []   # (cls, col0_in_class, ncols, s16off)
        s16 = 0
        for cls in range(2):
            done = 0
            while done < wcls:
                nc_ = min(COLCAP, wcls - done)
                calls.append((cls, done, nc_, s16))
                s16 += nc_ * 8
                done += nc_
        tiles.append(dict(wl=wl, wh=wh, calls=calls, s16=s16,
                          s16base=s16base))
        s16base += s16
    S16TOT = max(s16base, 8)

    # ---- per-core idx16 + kbig in the global layout ----
    for c in range(NCORES):
        cc = cores[c]
        ks, indptr = cc["ks"], cc["indptr"]
        data = cc["csr_data"] if len(cc["csr_data"]) else np.zeros(1,
                                                                   np.int64)
        idx16 = np.zeros((P, S16TOT), np.int16)
        for t in range(ntiles):
            ti = tiles[t]
            wl = ti["wl"]
            dp = t * P + np.arange(P)
            for (cls, col0, ncols, s16off) in ti["calls"]:
                cols = col0 + np.arange(ncols)
                gsel = np.where(cols < wl, 2 * cls, 2 * cls + 1)
                jj = np.where(cols < wl, cols, cols - wl)
                cnt = ks[dp[:, None], gsel[None, :]]        # [P, ncols]
                start = indptr[dp[:, None] * 4 + gsel[None, :]]
                gidx = np.minimum(start + jj[None, :], len(data) - 1)
                vals = data[gidx]
                mat = np.where(jj[None, :] < cnt, vals, PADQ)  # [P, ncols]
                flat = mat.T.reshape(-1)                       # i = col*128+p
                wrapped = flat.reshape(-1, 16).T.astype(np.int16)
                idx16[:, ti["s16base"] + s16off:
                      ti["s16base"] + s16off + ncols * 8] = np.tile(
                          wrapped, (8, 1))
        cc["idx16"] = idx16

        # sum pad-correction: pads = 2*(wl+wh) - ktot  ->  +pads*BIG
        ktot = ks.sum(axis=1)
        wtot_n = (2 * (wLs + wHs))[:, None].repeat(P, axis=1).reshape(-1)
        kbig = ((wtot_n - ktot) * BIG).astype(_F32)
        cc["kbig"] = np.ascontiguousarray(
            kbig.reshape(ntiles, P).T)                     # [128, ntiles]

    return dict(cores=cores, NMAX=NMAX, ntiles=ntiles, tiles=tiles,
                S16TOT=S16TOT, node0=node0, ncs=ncs)


def _tabpos(n):
    """Table position of node n: class-major layout so each gather class
    is a contiguous [25600, 128]-bf16 (256B-row) region."""
    return ((n % 4) // 2) * (NTAB // 2) + (n // 4) * 2 + (n % 2)


def _make_inputs(cfg, node, W_bases, W_comb, b_comb, bias_out, gn_weight,
                 gn_bias, gn_mean_scale):
    node = np.asarray(node, _F32)
    NMAX = cfg["NMAX"]
    nodeTF = np.zeros((P, NTAB), BF16)
    nodeTF[:, _tabpos(np.arange(N))] = node.T.astype(BF16)
    wcat = np.concatenate([np.asarray(W_bases, _F32),
                           np.asarray(W_comb, _F32)], axis=1).astype(BF16)
    bcomb = np.asarray(b_comb, _F32).reshape(1, BF)
    gaux0 = np.zeros((GPAD, 520), _F32)
    gaux0[:, 1:129] = np.asarray(bias_out, _F32)[None, :]
    gaux0[:, 129:257] = np.asarray(gn_mean_scale, _F32)[None, :]
    gaux0[:, 257:385] = np.asarray(gn_weight, _F32)[None, :]
    gaux0[:, 385:513] = np.asarray(gn_bias, _F32)[None, :]

    in_maps = []
    for c in range(NCORES):
        cc = cfg["cores"][c]
        nl = np.zeros((NMAX, D), _F32)
        nl[:cc["n"]] = node[cc["node0"]:cc["node0"] + cc["n"]][cc["perm"]]
        ga = gaux0.copy()
        ga[:, 0] = cc["cntinv"]
        in_maps.append({
            "nodeTF": nodeTF,                              # [128, NTAB] bf16
            "nodeTL": np.ascontiguousarray(nl.T).astype(BF16),
            "wcat": wcat,                                  # [128, 128] bf16
            "bcomb": bcomb,                                # [1, 64] f32
            "idx": cc["idx16"],                            # [128, S16TOT]
            "kbig": cc["kbig"],                            # [128, ntiles] f32
            "ind": np.ascontiguousarray(cc["ind"]).astype(BF16),
            "indT": np.ascontiguousarray(cc["indT"]),      # [16, nt*128] f32
            "gaux": ga,                                    # [16, 520] f32
        })
    return in_maps


# ======================================================================
# numpy simulation of the device algorithm (with bf16 quantization)
# ======================================================================
def _numpy_sim(cfg, in_maps):
    outs = []
    ntiles, NMAX = cfg["ntiles"], cfg["NMAX"]
    im0 = in_maps[0]
    nodeTF = im0["nodeTF"].astype(_F32)
    wcat = im0["wcat"].astype(_F32)
    table = (nodeTF.T @ wcat[:, :BF]).astype(BF16)          # [NTAB, 64]
    table[_tabpos(np.arange(N, N + 4))] = BF16.type(-BIG)

    for c in range(NCORES):
        im = in_maps[c]
        nodeTL = im["nodeTL"].astype(_F32)
        full = nodeTL.T @ wcat                              # [NMAX, 128]
        basloc = full[:, :BF].astype(BF16)
        comb = (full[:, BF:] + im["bcomb"][0][None, :]).astype(BF16)

        kbig = im["kbig"].T                                 # [ntiles, 128]
        h0 = np.zeros((NMAX, D), _F32)
        for t in range(ntiles):
            ti = cfg["tiles"][t]
            wl, wh = ti["wl"], ti["wh"]
            wcls = wl + wh
            wtot = 2 * wcls
            msg = np.zeros((P, max(wtot, 1), 128), _F32)
            for (cls, col0, ncols, s16off) in ti["calls"]:
                blk = im["idx"][:16, ti["s16base"] + s16off:
                                ti["s16base"] + s16off + ncols * 8]
                f2 = blk.T.reshape(-1)
                vals = f2[:ncols * 128].astype(np.int64)
                # class c region row r -> table rows (c*NTAB/2 + 2r, +1)
                # = nodes (4r+2c, 4r+2c+1): 128 bf16 payload
                base = cls * (NTAB // 2)
                rows = np.concatenate(
                    [table[base + 2 * vals],
                     table[base + 2 * vals + 1]],
                    axis=1).astype(_F32)                    # [ncols*128,128]
                msg[:, cls * wcls + col0:cls * wcls + col0 + ncols, :] = \
                    rows.reshape(ncols, P, 128).transpose(1, 0, 2)
            mv = msg.reshape(P, 2, -1, 128)[:, :, :wcls or 1, :]
            bl = basloc[t * P:(t + 1) * P].astype(_F32)
            ssum = kbig[t][:, None] + bl
            smax = np.full((P, BF), -BIG, _F32)
            if wl:
                ssum = ssum + mv[:, :, :wl, 0:64].sum(axis=(1, 2))
                smax = np.maximum(smax, mv[:, :, :wl, 0:64].max(axis=(1, 2)))
            if wh:
                ssum = ssum + mv[:, :, wl:wcls, 64:128].sum(axis=(1, 2))
                smax = np.maximum(smax,
                                  mv[:, :, wl:wcls, 64:128].max(axis=(1, 2)))
            smax = np.maximum(smax, bl)
            aggcat = np.concatenate([ssum, smax], axis=1)   # [P, 128]
            cb = comb[t * P:(t + 1) * P].astype(_F32)       # [P, 64]
            prod = (cb.reshape(P, H, 8, 1) *
                    aggcat.reshape(P, 1, 8, FH)).astype(BF16).astype(_F32)
            h0[t * P:(t + 1) * P] = prod.sum(axis=2).reshape(P, D)
        h0q = h0.astype(BF16).astype(_F32)
        hsq = (h0q * h0q).astype(BF16).astype(_F32)

        # graphnorm (f32, as baseline)
        ind = im["ind"].astype(_F32).reshape(P, ntiles, GPAD)
        ga = im["gaux"]
        cntinv = ga[:, 0:1]
        bias_o = ga[:, 1:129]
        ms = ga[:, 129:257]
        gnw = ga[:, 257:385]
        gnb = ga[:, 385:513]
        s1 = np.zeros((GPAD, D), _F32)
        s2 = np.zeros((GPAD, D), _F32)
        for t in range(ntiles):
            s1 += ind[:, t, :].T @ h0q[t * P:(t + 1) * P]
            s2 += ind[:, t, :].T @ hsq[t * P:(t + 1) * P]
        m0 = s1 * cntinv
        mh = m0 + bias_o
        e2 = s2 * cntinv + bias_o * (2 * m0 + bias_o)
        c0 = mh * ms
        var = e2 - 2 * c0 * mh + c0 * c0
        rstd = 1.0 / np.sqrt(var + EPS)
        Pm = gnw * rstd
        Qm = (bias_o - c0) * Pm + gnb
        indT = im["indT"].reshape(GPAD, ntiles, P)
        hfin = np.zeros((NMAX, D), _F32)
        for t in range(ntiles):
            Pn = indT[:, t, :].T @ Pm
            Qn = indT[:, t, :].T @ Qm
            hfin[t * P:(t + 1) * P] = np.maximum(
                h0q[t * P:(t + 1) * P] * Pn + Qn, 0.0)
        outs.append(hfin.astype(BF16))
    return outs


def _assemble(cfg, per_core_h):
    out = np.zeros((N, D), _F32)
    for c in range(NCORES):
        cc = cfg["cores"][c]
        h = np.asarray(per_core_h[c])
        if h.dtype != _F32:
            h = h.astype(_F32)
        out[cc["node0"] + cc["perm"]] = h[:cc["n"]]
    return out


# ======================================================================
# device program
# ======================================================================
def _build(cfg):
    import concourse.bacc as bacc
    import concourse.tile as tile
    from concourse import mybir

    ntiles, NMAX, S16TOT = cfg["ntiles"], cfg["NMAX"], cfg["S16TOT"]
    f32 = mybir.dt.float32
    bf16 = mybir.dt.bfloat16
    i16 = mybir.dt.int16
    ALU = mybir.AluOpType
    ACT = mybir.ActivationFunctionType
    AX = mybir.AxisListType

    nc = bacc.Bacc("TRN2", target_bir_lowering=False, debug=False,
                   num_devices=NCORES, num_swdge_queues=4,
                   dynamic_dma_scratch_size=SCRATCH)

    nodeTF = nc.dram_tensor("nodeTF", [P, NTAB], bf16,
                            kind="ExternalInput").ap()
    nodeTL = nc.dram_tensor("nodeTL", [P, NMAX], bf16,
                            kind="ExternalInput").ap()
    wcat = nc.dram_tensor("wcat", [D, D], bf16, kind="ExternalInput").ap()
    bcomb = nc.dram_tensor("bcomb", [1, BF], f32, kind="ExternalInput").ap()
    idx = nc.dram_tensor("idx", [P, S16TOT], i16, kind="ExternalInput").ap()
    kbig = nc.dram_tensor("kbig", [P, ntiles], f32,
                          kind="ExternalInput").ap()
    ind = nc.dram_tensor("ind", [P, ntiles * GPAD], bf16,
                         kind="ExternalInput").ap()
    indT = nc.dram_tensor("indT", [GPAD, ntiles * P], f32,
                          kind="ExternalInput").ap()
    gaux = nc.dram_tensor("gaux", [GPAD, 520], f32, kind="ExternalInput").ap()
    h_out = nc.dram_tensor("h", [NMAX, D], bf16, kind="ExternalOutput").ap()

    with tile.TileContext(nc) as tc:
        with (
            tc.tile_pool(name="dram", bufs=1, space="DRAM") as dram,
            tc.tile_pool(name="persist", bufs=1) as pp,
            tc.tile_pool(name="work", bufs=3) as wp,
            tc.tile_pool(name="nstream", bufs=2) as nsp,
            tc.tile_pool(name="idxp", bufs=6) as ixp,
            tc.tile_pool(name="msgp", bufs=3) as mp,
            tc.tile_pool(name="psum", bufs=2, space="PSUM") as psp,
            tc.tile_pool(name="psumb", bufs=3, space="PSUM") as psb,
            tc.tile_pool(name="statps", bufs=1, space="PSUM") as stp,
            tc.tile_pool(name="warm", bufs=1) as wmp,
        ):
            tbl0 = dram.tile([NTAB // 2, BF], bf16)
            tbl1 = dram.tile([NTAB // 2, BF], bf16)
            tblh = [tbl0, tbl1]

            # ---- constants / persistent
            wcat_s = pp.tile([D, D], bf16)
            nc.sync.dma_start(wcat_s[:], wcat[:])
            bcomb_s = pp.tile([1, BF], f32)
            nc.sync.dma_start(bcomb_s[:], bcomb[:])
            ones1 = pp.tile([1, P], f32)
            nc.vector.memset(ones1[:], 1.0)
            padt = pp.tile([2, BF], bf16)
            nc.vector.memset(padt[:], -BIG)
            kbig_s = pp.tile([P, ntiles], f32)
            nc.sync.dma_start(kbig_s[:], kbig[:])
            ind_s = pp.tile([P, ntiles * GPAD], bf16)
            nc.sync.dma_start(ind_s[:], ind[:])
            gaux_s = pp.tile([GPAD, 520], f32)
            nc.sync.dma_start(gaux_s[:], gaux[:])

            comb_all = pp.tile([P, ntiles * BF], bf16)
            basloc = pp.tile([P, ntiles * BF], bf16)
            h0_all = pp.tile([P, ntiles * D], bf16)

            # warmup: force the gpsimd gather-library load + queue setup
            # before phase A so the first real gather isn't delayed.
            # NB: lives in the LAST pool so it can't shift the 256B alignment
            # of the msg/idx pools (a 272B shift cost ~35% gather bandwidth).
            wupi = wmp.tile([P, 8], i16)
            nc.vector.memset(wupi[:], 0)
            wupm = wmp.tile([P, 1, 128], bf16)
            nc.gpsimd.dma_gather(
                wupm[:], nodeTF[:1, :].rearrange("o (r f) -> (o r) f", f=128),
                wupi[:], P, P, 128, queue_num=0, single_packet=True)

            # ---------------- phase A-full: global bases table ------------
            NFT = NTAB // P           # 800 table tiles
            AF = 8                    # nodeTF stream chunk
            TW = 8                    # table-write batch (tiles)
            for t in range(NFT):
                if t % AF == 0:
                    nfb = nsp.tile([P, AF * P], bf16, tag="nfb")
                    nb = min(AF, NFT - t)
                    nc.sync.dma_start(nfb[:, :nb * P],
                                      nodeTF[:, t * P:(t + nb) * P])
                if t % TW == 0:
                    bw = nsp.tile([P, TW * BF], bf16, tag="bw")
                nt_ = nfb[:, (t % AF) * P:(t % AF + 1) * P]
                psB = psb.tile([P, BF], f32, tag="psB")
                nc.tensor.matmul(psB[:], nt_, wcat_s[:, :BF],
                                 start=True, stop=True)
                # alternate copy engine: Act / DVE (both idle in phase A)
                if t % 2 == 0:
                    nc.scalar.copy(bw[:, (t % TW) * BF:(t % TW + 1) * BF],
                                   psB[:])
                else:
                    nc.vector.tensor_copy(
                        bw[:, (t % TW) * BF:(t % TW + 1) * BF], psB[:])
                if t % TW == TW - 1:
                    t0 = t - (TW - 1)
                    half, hr = divmod(t0 * P, NTAB // 2)
                    out_ap = tblh[half][hr:hr + TW * P, :].rearrange(
                        "(a p) f -> p a f", p=P)
                    nc.sync.dma_start(out_ap, bw[:])
                    if (t + 1) * P == (half + 1) * (NTAB // 2):
                        # half complete: write its -BIG pad rows now so
                        # gathers on this class aren't gated on the rest
                        nc.sync.dma_start(
                            tblh[half][2 * PADQ:2 * PADQ + 2, :], padt[:])

            # ---------------- phase A-local: bases+comb (dst-sorted) ------
            AL = 10
            for t in range(ntiles):
                if t % AL == 0:
                    nlb = nsp.tile([P, AL * P], bf16, tag="nlb")
                    nb = min(AL, ntiles - t)
                    nc.sync.dma_start(nlb[:, :nb * P],
                                      nodeTL[:, t * P:(t + nb) * P])
                nt_ = nlb[:, (t % AL) * P:(t % AL + 1) * P]
                psA = psp.tile([P, D], f32, tag="psA")
                nc.tensor.matmul(psA[:], nt_, wcat_s[:], start=True,
                                 stop=False)
                nc.tensor.matmul(psA[:, BF:], ones1[:], bcomb_s[:],
                                 start=False, stop=True)
                nc.scalar.copy(basloc[:, t * BF:(t + 1) * BF], psA[:, :BF])
                nc.scalar.copy(comb_all[:, t * BF:(t + 1) * BF], psA[:, BF:])

            # gather source: two contiguous class regions, 256B rows
            tblq = [tblh[c][:].rearrange("(r x) f -> r (x f)", x=2)
                    for c in range(2)]

            # ---------------- phase C: gather + aggregate + einsum --------
            stats = stp.tile([GPAD, 2 * D], f32)
            qrot = 0
            first_mm = True
            for zi, t in enumerate(range(ntiles)):
                ti = cfg["tiles"][t]
                wl, wh = ti["wl"], ti["wh"]
                wcls = wl + wh
                wtot = 2 * wcls
                idxt = ixp.tile([P, max(ti["s16"], 8)], i16, tag="idxt")
                if ti["s16"]:
                    nc.sync.dma_start(
                        idxt[:, :ti["s16"]],
                        idx[:, ti["s16base"]:ti["s16base"] + ti["s16"]])
                msg = mp.tile([P, max(wtot, 1), 128], bf16, tag="msg")
                for (cls, col0, ncols, s16off) in ti["calls"]:
                    nc.gpsimd.dma_gather(
                        msg[:, cls * wcls + col0:cls * wcls + col0 + ncols,
                            :],
                        tblq[cls],
                        idxt[:, s16off:s16off + ncols * 8],
                        ncols * P, ncols * P, 128,
                        queue_num=qrot % 4,
                        single_packet=True,
                    )
                    qrot += 1
                # views: [p, cls, col, elem]; all 4 reduces first so the
                # msg buffer frees as early as possible
                mv = msg[:].rearrange("p (c w) e -> p c w e", c=2)
                aggcat = wp.tile([P, 2 * BF], f32, tag="aggcat")
                asum = wp.tile([P, BF], f32, tag="asum")
                amax = wp.tile([P, BF], f32, tag="amax")
                if wl > 0:
                    nc.vector.tensor_reduce(
                        asum[:],
                        mv[:, :, :wl, 0:BF].rearrange("p c w e -> p e c w"),
                        axis=AX.XY, op=ALU.add)
                    nc.vector.tensor_reduce(
                        amax[:],
                        mv[:, :, :wl, 0:BF].rearrange("p c w e -> p e c w"),
                        axis=AX.XY, op=ALU.max)
                else:
                    nc.vector.memset(asum[:], 0.0)
                    nc.vector.memset(amax[:], -BIG)
                if wh > 0:
                    nc.vector.tensor_reduce(
                        aggcat[:, :BF],
                        mv[:, :, wl:wcls, BF:2 * BF].rearrange(
                            "p c w e -> p e c w"),
                        axis=AX.XY, op=ALU.add)
                    nc.vector.tensor_reduce(
                        aggcat[:, BF:],
                        mv[:, :, wl:wcls, BF:2 * BF].rearrange(
                            "p c w e -> p e c w"),
                        axis=AX.XY, op=ALU.max)
                    nc.vector.tensor_tensor(out=asum[:], in0=asum[:],
                                            in1=aggcat[:, :BF], op=ALU.add)
                    nc.vector.tensor_tensor(out=amax[:], in0=amax[:],
                                            in1=aggcat[:, BF:], op=ALU.max)
                # + pad correction, then + self-loop bases
                nc.scalar.activation(asum[:], asum[:], ACT.Identity,
                                     bias=kbig_s[:, t:t + 1], scale=1.0)
                nc.vector.tensor_tensor(out=aggcat[:, :BF], in0=asum[:],
                                        in1=basloc[:, t * BF:(t + 1) * BF],
                                        op=ALU.add)
                nc.vector.tensor_tensor(out=aggcat[:, BF:], in0=amax[:],
                                        in1=basloc[:, t * BF:(t + 1) * BF],
                                        op=ALU.max)
                # einsum: h0[p, h*16] = sum_k comb[p,h,k] * aggcat[p,k,16]
                prod = wp.tile([P, H, 8, FH], bf16, tag="prod")
                cview = comb_all[:, t * BF:(t + 1) * BF].rearrange(
                    "p (h k) -> p h k", h=H)
                nc.vector.tensor_tensor(
                    out=prod[:],
                    in0=cview.to_broadcast([P, H, 8, FH]),
                    in1=aggcat[:].rearrange("p (k f) -> p k f", k=8)
                    [:, None, :, :].broadcast_to([P, H, 8, FH]),
                    op=ALU.mult)
                h0f = wp.tile([P, D], f32, tag="h0f")
                nc.vector.tensor_reduce(
                    h0f[:], prod[:].rearrange("p h k f -> p h f k"),
                    axis=AX.X, op=ALU.add)
                nc.scalar.copy(h0_all[:, t * D:(t + 1) * D], h0f[:])
                hsq = wp.tile([P, D], bf16, tag="hsq")
                nc.scalar.square(hsq[:], h0f[:])
                nc.tensor.matmul(
                    stats[:, :D], ind_s[:, t * GPAD:(t + 1) * GPAD],
                    h0_all[:, t * D:(t + 1) * D],
                    start=first_mm, stop=(zi == ntiles - 1))
                nc.tensor.matmul(
                    stats[:, D:], ind_s[:, t * GPAD:(t + 1) * GPAD],
                    hsq[:],
                    start=first_mm, stop=(zi == ntiles - 1))
                first_mm = False

            # ---------------- phase D: per-graph P/Q ----------------
            st = pp.tile([GPAD, 2 * D], f32)
            nc.vector.tensor_copy(st[:], stats[:])
            cntinv = gaux_s[:, 0:1]
            bias_o = gaux_s[:, 1:129]
            ms = gaux_s[:, 129:257]
            gnw = gaux_s[:, 257:385]
            gnb = gaux_s[:, 385:513]
            s1 = st[:, :D]
            s2 = st[:, D:]
            m0 = pp.tile([GPAD, D], f32)
            nc.vector.tensor_scalar_mul(m0[:], s1, cntinv)
            mh = pp.tile([GPAD, D], f32)
            nc.vector.tensor_tensor(out=mh[:], in0=m0[:], in1=bias_o,
                                    op=ALU.add)
            t1 = pp.tile([GPAD, D], f32)
            nc.vector.scalar_tensor_tensor(out=t1[:], in0=m0[:], scalar=2.0,
                                           in1=bias_o, op0=ALU.mult,
                                           op1=ALU.add)
            t2 = pp.tile([GPAD, D], f32)
            nc.vector.tensor_tensor(out=t2[:], in0=bias_o, in1=t1[:],
                                    op=ALU.mult)
            e2 = pp.tile([GPAD, D], f32)
            nc.vector.tensor_scalar_mul(e2[:], s2, cntinv)
            nc.vector.tensor_tensor(out=e2[:], in0=e2[:], in1=t2[:],
                                    op=ALU.add)
            c0 = pp.tile([GPAD, D], f32)
            nc.vector.tensor_tensor(out=c0[:], in0=mh[:], in1=ms,
                                    op=ALU.mult)
            t3 = pp.tile([GPAD, D], f32)
            nc.vector.tensor_tensor(out=t3[:], in0=c0[:], in1=mh[:],
                                    op=ALU.mult)
            var = pp.tile([GPAD, D], f32)
            nc.vector.scalar_tensor_tensor(out=var[:], in0=t3[:],
                                           scalar=-2.0, in1=e2[:],
                                           op0=ALU.mult, op1=ALU.add)
            t4 = pp.tile([GPAD, D], f32)
            nc.vector.tensor_tensor(out=t4[:], in0=c0[:], in1=c0[:],
                                    op=ALU.mult)
            nc.vector.tensor_tensor(out=var[:], in0=var[:], in1=t4[:],
                                    op=ALU.add)
            stdv = pp.tile([GPAD, D], f32)
            epsc = pp.tile([GPAD, 1], f32)
            nc.vector.memset(epsc[:], EPS)
            nc.scalar.activation(stdv[:], var[:], ACT.Sqrt, bias=epsc[:],
                                 scale=1.0)
            rstd = pp.tile([GPAD, D], f32)
            nc.vector.reciprocal(rstd[:], stdv[:])
            PQ = pp.tile([GPAD, 2 * D], f32)
            nc.vector.tensor_tensor(out=PQ[:, :D], in0=gnw, in1=rstd[:],
                                    op=ALU.mult)
            t5 = pp.tile([GPAD, D], f32)
            nc.vector.tensor_tensor(out=t5[:], in0=bias_o, in1=c0[:],
                                    op=ALU.subtract)
            nc.vector.tensor_tensor(out=PQ[:, D:], in0=t5[:], in1=PQ[:, :D],
                                    op=ALU.mult)
            nc.vector.tensor_tensor(out=PQ[:, D:], in0=PQ[:, D:], in1=gnb,
                                    op=ALU.add)

            # ------- phase E: normalize + relu + out (2-tile batches) -----
            ET = 10
            EB = 2
            for t in range(0, ntiles, EB):
                if t % ET == 0:
                    itb = nsp.tile([GPAD, ET * P], f32, tag="itb")
                    nb = min(ET, ntiles - t)
                    nc.sync.dma_start(itb[:, :nb * P],
                                      indT[:, t * P:(t + nb) * P])
                pq = psp.tile([P, EB, 2 * D], f32, tag="pq")
                for b in range(EB):
                    nc.tensor.matmul(
                        pq[:, b, :],
                        itb[:, (t % ET + b) * P:(t % ET + b + 1) * P],
                        PQ[:], start=True, stop=True)
                hf = wp.tile([P, EB, D], f32, tag="hf")
                nc.vector.tensor_tensor(
                    out=hf[:],
                    in0=h0_all[:, t * D:(t + EB) * D].rearrange(
                        "p (b d) -> p b d", b=EB),
                    in1=pq[:, :, :D], op=ALU.mult)
                nc.vector.tensor_tensor(out=hf[:], in0=hf[:],
                                        in1=pq[:, :, D:], op=ALU.add)
                ho = wp.tile([P, EB, D], bf16, tag="ho")
                nc.scalar.activation(ho[:], hf[:], ACT.Relu)
                nc.sync.dma_start(
                    h_out[t * P:(t + EB) * P, :].rearrange(
                        "(a p) f -> p a f", p=P),
                    ho[:])

    nc.compile()
    return nc


_CACHE = {}


def kernel(node, edge_index, edge_attr, batch_ptr, W_bases, W_comb, b_comb,
           bias_out, gn_weight, gn_bias, gn_mean_scale):
    node = np.asarray(node)
    edge_index = np.asarray(edge_index)
    batch_ptr = np.asarray(batch_ptr)
    cfg = _prep(edge_index, batch_ptr)
    in_maps = _make_inputs(cfg, node, W_bases, W_comb, b_comb, bias_out,
                           gn_weight, gn_bias, gn_mean_scale)

    if os.environ.get("EGC_NUMPY_SIM"):
        return _assemble(cfg, _numpy_sim(cfg, in_maps))

    from concourse.bass_utils import run_bass_kernel_spmd
    key = "prog"
    if key not in _CACHE:
        _CACHE[key] = _build(cfg)
    nc = _CACHE[key]
    res = run_bass_kernel_spmd(nc, in_maps, core_ids=list(range(NCORES)),
                               **_CACHE.get("run_kwargs", {}))
    _CACHE["last_res"] = res
    return _assemble(cfg, [res.results[c]["h"] for c in range(NCORES)])
